# revision 14
# baseline (speedup 1.0000x reference)
"""EntityAttentionLayer Trainium2 kernel (v2, all-bf16 datapath).

Data-parallel over batch across 8 NeuronCores (256 batches/core).
Per core, per G-group of 16 batches (1024 entity tokens, 256 query tokens),
software-pipelined so group g's attention runs while group g+1's K/Q
projections keep the PE busy:

  xt:    entities, feature-on-partition, bf16        [128, 4dc, 1024]
  kf:    K feature-major GEMM -> bf16                [128, 4c, 1024]
  vt:    V token-major GEMM -> bf16                  [128, 8t, 512]
  bdq:   Q gathered from xt via strided moving AP, written block-diagonal
         over head-parity                            [128, 4c, 8g2, 64]
  logits: per (c, g2): kf[128,128] stationary (2 batches) x bdq cols
         -> psum [128 tok, 8g2, 64]; junk cross-batch cells + pre-mask
         handled by ONE identity-stationary matmul adding mneg (-1e30)
  exp:   ACT -> bde bf16 (masked cells exp -> 0)     [128, 4c, 8g2, 64]
  sums:  all-ones [128,128] stationary matmul -> denominators replicated
         across all 128 partitions; reciprocal_approx_fast -> rs f32
  attnv: vt stationary x bde moving -> psum with junk parity blocks;
         diagonal blocks extracted * rs (fused normalize) -> ao bf16
  oproj: ao stationary x woT moving; post-mask applied via ACT Copy with
         per-partition scale; DMA out f32
"""

import os
import sys

import numpy as np

sys.path.insert(0, "/opt/trn_rl_repo")

import concourse.bass as bass
import concourse.mybir as mybir
import concourse.tile as tile
from concourse import bacc, bass_utils

import ml_dtypes

F32 = mybir.dt.float32
BF16 = mybir.dt.bfloat16
AF = mybir.ActivationFunctionType
ALU = mybir.AluOpType

BS, NE, NQ = 2048, 64, 16
D = 512
H = 8
HD = 64
NCORES = 8
BPC = BS // NCORES          # 256 batches per core
GB = 16                     # batches per G-group
NG = BPC // GB              # 16 groups
NTOK = BPC * NE             # 16384 entity tokens per core
NQT = BPC * NQ              # 4096 query tokens per core


def build_nc(debug=False):
    nc = bacc.Bacc()
    dbg = {}
    if debug:
        for nm, shp, dt in [
            ("dxt", [128, 4096], BF16), ("dkf", [128, 4096], BF16),
            ("dbdq", [128, 2048], BF16), ("dbde", [128, 2048], BF16),
            ("dvt", [128, 4096], BF16), ("drs", [128, 2048], F32),
            ("dao", [128, 1024], BF16),
        ]:
            dbg[nm] = nc.declare_dram_parameter(nm, shp, dt, isOutput=True)

    ent = nc.declare_dram_parameter("ent", [D, NTOK], BF16, isOutput=False)
    msk = nc.declare_dram_parameter("msk", [128, NG * 512], BF16, isOutput=False)
    pmt = nc.declare_dram_parameter("pmt", [NQT], F32, isOutput=False)
    wqT = nc.declare_dram_parameter("wqT", [D, 512], BF16, isOutput=False)
    wkT = nc.declare_dram_parameter("wkT", [D, 512], BF16, isOutput=False)
    wvT = nc.declare_dram_parameter("wvT", [D, 512], BF16, isOutput=False)
    woT = nc.declare_dram_parameter("woT", [512, 512], BF16, isOutput=False)
    out = nc.declare_dram_parameter("out", [NQT, 512], F32, isOutput=True)

    ent_r = ent.rearrange("(dc p) n -> p dc n", p=128)   # [128, 4, 16384]
    pmt_r = pmt.rearrange("(o p) -> p o", p=128)         # [128, 32]

    with tile.TileContext(nc) as tc:
        with (
            tc.tile_pool(name="wpool", bufs=1) as wpool,
            tc.tile_pool(name="xtp", bufs=2) as xtp,
            tc.tile_pool(name="kfp", bufs=2) as kfp,
            tc.tile_pool(name="vtp", bufs=2) as vtp,
            tc.tile_pool(name="persist", bufs=1) as persist,
            tc.tile_pool(name="bdep", bufs=2) as bdep,
            tc.tile_pool(name="mnp", bufs=2) as mnp,
            tc.tile_pool(name="rsp", bufs=2) as rsp,
            tc.tile_pool(name="aop", bufs=2) as aop,
            tc.tile_pool(name="osp", bufs=2) as osp,
            tc.tile_pool(name="psp", bufs=2, space="PSUM") as psp,
            tc.tile_pool(name="pslp", bufs=2, space="PSUM") as pslp,
            tc.tile_pool(name="avp", bufs=2, space="PSUM") as avp,
        ):
            # ---- constants / weights (loaded once) ----
            wq_t = wpool.tile([128, 4, 512], BF16, tag="wq")
            wk_t = wpool.tile([128, 4, 512], BF16, tag="wk")
            wv_t = wpool.tile([128, 4, 512], BF16, tag="wv")
            wo_t = wpool.tile([128, 4, 512], BF16, tag="wo")
            nc.gpsimd.dma_start(wk_t, wkT.rearrange("(dc p) e -> p dc e", p=128))
            nc.gpsimd.dma_start(wq_t, wqT.rearrange("(dc p) e -> p dc e", p=128))
            nc.gpsimd.dma_start(wv_t, wvT.rearrange("(dc p) e -> p dc e", p=128))
            nc.gpsimd.dma_start(wo_t, woT.rearrange("(ec p) o -> p ec o", p=128))
            pm_t = wpool.tile([128, 32], F32, tag="pm")
            nc.gpsimd.dma_start(pm_t, pmt_r)
            ones_t = wpool.tile([128, 128], BF16, tag="ones")
            nc.vector.memset(ones_t, 1.0)

            # persistent block-diagonal Q (off-parity blocks stay zero)
            bdqs = [
                persist.tile([128, 4, 8, 64], BF16, tag=f"bdq{i}", name=f"bdq{i}")
                for i in range(2)
            ]
            for t in bdqs:
                nc.vector.memset(t, 0.0)

            xts = {}
            kfs = {}
            mns = {}
            pending_o = []

            def oproj_flush():
                while pending_o:
                    og, oao = pending_o.pop(0)
                    for tc2 in range(2):
                        ps_o = psp.tile([128, 512], F32, tag="ps", name="ps_o")
                        for ec in range(4):
                            nc.tensor.matmul(
                                ps_o,
                                oao[:, ec, tc2 * 128 : (tc2 + 1) * 128],
                                wo_t[:, ec, :],
                                start=(ec == 0), stop=(ec == 3),
                            )
                        out_s = osp.tile([128, 512], F32, tag="outs", name="out_s")
                        nc.scalar.activation(
                            out_s, ps_o, AF.Copy,
                            scale=pm_t[:, og * 2 + tc2 : og * 2 + tc2 + 1],
                        )
                        q0 = og * GB * NQ
                        nc.sync.dma_start(
                            out[q0 + tc2 * 128 : q0 + (tc2 + 1) * 128, :], out_s
                        )

            def load_xt(g):
                xt = xtp.tile([128, 4, GB * NE], BF16, tag="xt", name=f"xt{g}")
                t0 = g * GB * NE
                for h in range(2):
                    nc.sync.dma_start(
                        xt[:, :, h * 512 : (h + 1) * 512],
                        ent_r[:, :, t0 + h * 512 : t0 + (h + 1) * 512],
                    )
                xts[g] = xt

            def load_mn(g):
                mn = mnp.tile([128, 8, 64], BF16, tag="mn", name=f"mn{g}")
                nc.sync.dma_start(
                    mn, msk.rearrange("p (g x) -> p g x", g=NG)[:, g, :]
                )
                mns[g] = mn

            def kproj(g):
                """K projection for group g -> kf[g] (bf16)."""
                xt = xts[g]
                kf = kfp.tile([128, 4, GB * NE], BF16, tag="kf", name=f"kf{g}")
                for ec in range(4):
                    for fg in range(2):
                        ps_k = psp.tile([128, 512], F32, tag="ps", name="ps_k")
                        for dc in range(4):
                            nc.tensor.matmul(
                                ps_k,
                                wk_t[:, dc, ec * 128 : (ec + 1) * 128],
                                xt[:, dc, fg * 512 : (fg + 1) * 512],
                                start=(dc == 0), stop=(dc == 3),
                            )
                        dst = kf[:, ec, fg * 512 : (fg + 1) * 512]
                        if ec < 2:
                            nc.scalar.copy(dst, ps_k)
                        else:
                            nc.vector.tensor_copy(dst, ps_k)
                kfs[g] = kf

            def qproj(g):
                """Q projection for group g -> block-diagonal bdq[g%2]."""
                xt = xts[g]
                bdq = bdqs[g % 2]
                xq_view = xt.rearrange("p dc (b t) -> p dc b t", b=GB)
                for eh in range(2):
                    ps_q = psp.tile([128, 2, 256], F32, tag="ps", name="ps_q")
                    for ei in range(2):
                        ec = eh * 2 + ei
                        for dc in range(4):
                            nc.tensor.matmul(
                                ps_q[:, ei, :],
                                wq_t[:, dc, ec * 128 : (ec + 1) * 128],
                                xq_view[:, dc, :, 0:NQ],
                                start=(dc == 0), stop=(dc == 3),
                            )
                    ps_qv = ps_q.rearrange("p c (g2 x) -> p c g2 x", g2=8)
                    cs = slice(eh * 2, eh * 2 + 2)
                    nc.scalar.copy(bdq[0:64, cs, :, 0:32], ps_qv[0:64])
                    nc.scalar.copy(bdq[64:128, cs, :, 32:64], ps_qv[64:128])

            # ---- prologue: group 0's K/Q + first mask ----
            load_xt(0)
            load_mn(0)
            kproj(0)
            qproj(0)

            for g in range(NG):
                xt = xts.pop(g)
                kf = kfs.pop(g)
                mn = mns.pop(g)
                bdq = bdqs[g % 2]

                # ---- logits + mask + exp -> bde ----
                bde = bdep.tile([128, 4, 8, 64], BF16, tag="bde", name=f"bde{g}")
                for c in range(4):
                    ps_l = pslp.tile([128, 8, 64], F32, tag="psl", name="ps_l")
                    for g2 in range(8):
                        nc.tensor.matmul(
                            ps_l[:, g2, :],
                            kf[:, c, g2 * 128 : (g2 + 1) * 128],
                            bdq[:, c, g2, :],
                            start=True, stop=True,
                        )
                    nc.scalar.activation(
                        bde[:, c, :, :], ps_l, AF.Exp, scale=1.0 / np.sqrt(HD)
                    )
                    # zero masked + cross-batch junk cells on the idle Pool engine
                    nc.gpsimd.tensor_tensor(
                        bde[:, c, :, :], bde[:, c, :, :], mn, ALU.mult
                    )

                if debug and g == 0:
                    nc.sync.dma_start(dbg["dxt"].rearrange("a b -> a b"), xt.rearrange("p a b -> p (a b)"))
                    nc.sync.dma_start(dbg["dkf"].rearrange("a b -> a b"), kf.rearrange("p a b -> p (a b)"))
                    nc.sync.dma_start(dbg["dbdq"].rearrange("a b -> a b"), bdq.rearrange("p a b c -> p (a b c)"))
                    nc.sync.dma_start(dbg["dbde"].rearrange("a b -> a b"), bde.rearrange("p a b c -> p (a b c)"))

                # ---- previous group's output projection (covers extract) ----
                oproj_flush()

                # ---- V projection (overlaps exp on ACT) ----
                vt = vtp.tile([128, 8, 512], BF16, tag="vt", name=f"vt{g}")
                for t8 in range(8):
                    ps_v = psp.tile([128, 512], F32, tag="ps", name="ps_v")
                    for dc in range(4):
                        nc.tensor.matmul(
                            ps_v,
                            xt[:, dc, t8 * 128 : (t8 + 1) * 128],
                            wv_t[:, dc, :],
                            start=(dc == 0), stop=(dc == 3),
                        )
                    if t8 < 4:
                        nc.scalar.copy(vt[:, t8, :], ps_v)
                    else:
                        nc.vector.tensor_copy(vt[:, t8, :], ps_v)

                # ---- softmax denominators (replicated across partitions) ----
                rs = rsp.tile([128, 4, 512], F32, tag="rs", name=f"rs{g}")
                for c in range(4):
                    ps_s = pslp.tile([128, 512], F32, tag="psl", name="ps_s")
                    nc.tensor.matmul(
                        ps_s,
                        ones_t,
                        bde[:, c, :, :].rearrange("p a b -> p (a b)"),
                        start=True, stop=True,
                    )
                    nc.vector.reciprocal_approx_fast(out=rs[:, c, :], in_=ps_s)

                if debug and g == 0:
                    nc.sync.dma_start(dbg["drs"].rearrange("a b -> a b"), rs.rearrange("p a b -> p (a b)"))
                    nc.sync.dma_start(dbg["dvt"].rearrange("a b -> a b"), vt.rearrange("p a b -> p (a b)"))

                # ---- attn @ V with junk parity blocks; extract diag * rs ----
                ao = aop.tile([128, 4, 256], BF16, tag="ao", name=f"ao{g}")
                ao_v = ao.rearrange("p c (h2 gi x) -> p c h2 gi x", h2=2, gi=4)
                rs_v = rs.rearrange("p c (g2 h2 x) -> p c g2 h2 x", g2=8, h2=2)
                for half in range(2):
                    av = avp.tile([128, 4, 4, 64], F32, tag="av", name="ps_av")
                    for gi in range(4):
                        g2 = half * 4 + gi
                        for c in range(4):
                            nc.tensor.matmul(
                                av[:, gi, c, :],
                                vt[:, g2, c * 128 : (c + 1) * 128],
                                bde[:, c, g2, :],
                                start=True, stop=True,
                            )
                    av_v = av.rearrange("p gi c x -> p c gi x")
                    for P in range(2):
                        psl = slice(P * 64, (P + 1) * 64)
                        nc.vector.tensor_tensor(
                            ao_v[psl, :, half, :, :],
                            av_v[psl, :, :, P * 32 : (P + 1) * 32],
                            rs_v[psl, :, half * 4 : (half + 1) * 4, P, :],
                            ALU.mult,
                        )

                if debug and g == 0:
                    nc.sync.dma_start(dbg["dao"].rearrange("a b -> a b"), ao.rearrange("p a b -> p (a b)"))

                pending_o.append((g, ao))

                # ---- next group's K/Q keep the PE busy while DVE/ACT drain ----
                if g + 1 < NG:
                    load_xt(g + 1)
                    load_mn(g + 1)
                    kproj(g + 1)
                    qproj(g + 1)
                else:
                    oproj_flush()

    nc.finalize()
    return nc


_NC_CACHE = None
RUN_KWARGS = {}
LAST_RESULT = None


def _get_nc():
    global _NC_CACHE
    if _NC_CACHE is None:
        _NC_CACHE = build_nc()
    return _NC_CACHE


def _bf16(x):
    return np.ascontiguousarray(x.astype(ml_dtypes.bfloat16))


def kernel(entities, pre_mask, post_mask, W_in, W_out, b_out):
    entities = np.asarray(entities, dtype=np.float32)
    pre_mask = np.asarray(pre_mask)
    post_mask = np.asarray(post_mask)
    W_in = np.asarray(W_in, dtype=np.float32)
    W_out = np.asarray(W_out, dtype=np.float32)
    b_out = np.asarray(b_out, dtype=np.float32)

    wqT = _bf16(W_in[0:512].T)
    wkT = _bf16(W_in[512:1024].T)
    wvT = _bf16(W_in[1024:1536].T)
    woT = _bf16(W_out.T)

    bp_idx = np.arange(2).reshape(2, 1, 1, 1, 1, 1, 1)
    B_idx = np.arange(2).reshape(1, 1, 1, 1, 1, 2, 1)

    in_maps = []
    for i in range(NCORES):
        bsl = slice(i * BPC, (i + 1) * BPC)
        ent_i = _bf16(entities[bsl].reshape(NTOK, D).T)
        # mneg[(bp,j), (g,g2,P,B,q)]: -1e30 where cross-batch or pre-masked
        pm_i = pre_mask[bsl, :NQ, :]                       # (256, 16, 64)
        pm_r = pm_i.reshape(NG, 8, 2, NQ, NE)              # (g, g2, B, q, j)
        pmx = pm_r.transpose(4, 0, 1, 2, 3)                # (j, g, g2, B, q)
        cond = bp_idx != B_idx                             # (2,1,1,1,1,2,1)
        cond = cond | pmx[None, :, :, :, None, :, :]       # (2,j,g,g2,P,B,q)
        cond = np.broadcast_to(cond, (2, NE, NG, 8, 2, 2, NQ))
        msk_i = _bf16(np.where(cond, 0.0, 1.0).reshape(128, NG * 512))
        pmt_i = np.ascontiguousarray(
            (1.0 - post_mask[bsl].astype(np.float32)).reshape(NQT)
        )
        in_maps.append(
            {
                "ent": ent_i,
                "msk": msk_i,
                "pmt": pmt_i,
                "wqT": wqT,
                "wkT": wkT,
                "wvT": wvT,
                "woT": woT,
            }
        )

    nc = _get_nc()
    res = bass_utils.run_bass_kernel_spmd(
        nc, in_maps, list(range(NCORES)), **RUN_KWARGS
    )
    global LAST_RESULT
    LAST_RESULT = res
    outs = [res.results[i]["out"].reshape(BPC, NQ, 512) for i in range(NCORES)]
    full = np.concatenate(outs, axis=0)
    if b_out.any():
        full = full + b_out[None, None, :]
        full = np.where(post_mask[:, :, None], 0.0, full)
    return full.astype(np.float32)


# revision 15
# speedup vs baseline: 1.0134x; 1.0134x over previous
"""EntityAttentionLayer Trainium2 kernel (v2, all-bf16 datapath).

Data-parallel over batch across 8 NeuronCores (256 batches/core).
Per core, per G-group of 16 batches (1024 entity tokens, 256 query tokens),
software-pipelined so group g's attention runs while group g+1's K/Q
projections keep the PE busy:

  xt:    entities, feature-on-partition, bf16        [128, 4dc, 1024]
  kf:    K feature-major GEMM -> bf16                [128, 4c, 1024]
  vt:    V token-major GEMM -> bf16                  [128, 8t, 512]
  bdq:   Q gathered from xt via strided moving AP, written block-diagonal
         over head-parity                            [128, 4c, 8g2, 64]
  logits: per (c, g2): kf[128,128] stationary (2 batches) x bdq cols
         -> psum [128 tok, 8g2, 64]; junk cross-batch cells + pre-mask
         handled by ONE identity-stationary matmul adding mneg (-1e30)
  exp:   ACT -> bde bf16 (masked cells exp -> 0)     [128, 4c, 8g2, 64]
  sums:  all-ones [128,128] stationary matmul -> denominators replicated
         across all 128 partitions; reciprocal_approx_fast -> rs f32
  attnv: vt stationary x bde moving -> psum with junk parity blocks;
         diagonal blocks extracted * rs (fused normalize) -> ao bf16
  oproj: ao stationary x woT moving; post-mask applied via ACT Copy with
         per-partition scale; DMA out f32
"""

import os
import sys

import numpy as np

sys.path.insert(0, "/opt/trn_rl_repo")

import concourse.bass as bass
import concourse.mybir as mybir
import concourse.tile as tile
from concourse import bacc, bass_utils

import ml_dtypes

F32 = mybir.dt.float32
BF16 = mybir.dt.bfloat16
AF = mybir.ActivationFunctionType
ALU = mybir.AluOpType

BS, NE, NQ = 2048, 64, 16
D = 512
H = 8
HD = 64
NCORES = 8
BPC = BS // NCORES          # 256 batches per core
GB = 16                     # batches per G-group
NG = BPC // GB              # 16 groups
NTOK = BPC * NE             # 16384 entity tokens per core
NQT = BPC * NQ              # 4096 query tokens per core


def build_nc(debug=False):
    nc = bacc.Bacc()
    dbg = {}
    if debug:
        for nm, shp, dt in [
            ("dxt", [128, 4096], BF16), ("dkf", [128, 4096], BF16),
            ("dbdq", [128, 2048], BF16), ("dbde", [128, 2048], BF16),
            ("dvt", [128, 4096], BF16), ("drs", [128, 2048], F32),
            ("dao", [128, 1024], BF16),
        ]:
            dbg[nm] = nc.declare_dram_parameter(nm, shp, dt, isOutput=True)

    ent = nc.declare_dram_parameter("ent", [D, NTOK], BF16, isOutput=False)
    msk = nc.declare_dram_parameter("msk", [128, NG * 512], BF16, isOutput=False)
    pmt = nc.declare_dram_parameter("pmt", [NQT], F32, isOutput=False)
    wqT = nc.declare_dram_parameter("wqT", [D, 512], BF16, isOutput=False)
    wkT = nc.declare_dram_parameter("wkT", [D, 512], BF16, isOutput=False)
    wvT = nc.declare_dram_parameter("wvT", [D, 512], BF16, isOutput=False)
    woT = nc.declare_dram_parameter("woT", [512, 512], BF16, isOutput=False)
    out = nc.declare_dram_parameter("out", [NQT, 512], F32, isOutput=True)

    ent_r = ent.rearrange("(dc p) n -> p dc n", p=128)   # [128, 4, 16384]
    pmt_r = pmt.rearrange("(o p) -> p o", p=128)         # [128, 32]

    with tile.TileContext(nc) as tc:
        with (
            tc.tile_pool(name="wpool", bufs=1) as wpool,
            tc.tile_pool(name="xtp", bufs=2) as xtp,
            tc.tile_pool(name="kfp", bufs=2) as kfp,
            tc.tile_pool(name="vtp", bufs=2) as vtp,
            tc.tile_pool(name="persist", bufs=1) as persist,
            tc.tile_pool(name="bdep", bufs=2) as bdep,
            tc.tile_pool(name="mnp", bufs=2) as mnp,
            tc.tile_pool(name="rsp", bufs=2) as rsp,
            tc.tile_pool(name="aop", bufs=2) as aop,
            tc.tile_pool(name="osp", bufs=2) as osp,
            tc.tile_pool(name="psp", bufs=2, space="PSUM") as psp,
            tc.tile_pool(name="pslp", bufs=2, space="PSUM") as pslp,
            tc.tile_pool(name="avp", bufs=2, space="PSUM") as avp,
        ):
            # ---- constants / weights (loaded once) ----
            wq_t = wpool.tile([128, 4, 512], BF16, tag="wq")
            wk_t = wpool.tile([128, 4, 512], BF16, tag="wk")
            wv_t = wpool.tile([128, 4, 512], BF16, tag="wv")
            wo_t = wpool.tile([128, 4, 512], BF16, tag="wo")
            nc.sync.dma_start(wk_t, wkT.rearrange("(dc p) e -> p dc e", p=128))
            nc.sync.dma_start(wq_t, wqT.rearrange("(dc p) e -> p dc e", p=128))
            nc.sync.dma_start(wv_t, wvT.rearrange("(dc p) e -> p dc e", p=128))
            nc.sync.dma_start(wo_t, woT.rearrange("(ec p) o -> p ec o", p=128))
            pm_t = wpool.tile([128, 32], F32, tag="pm")
            nc.sync.dma_start(pm_t, pmt_r)
            ones_t = wpool.tile([128, 128], BF16, tag="ones")
            nc.vector.memset(ones_t, 1.0)

            # persistent block-diagonal Q (off-parity blocks stay zero)
            bdqs = [
                persist.tile([128, 4, 8, 64], BF16, tag=f"bdq{i}", name=f"bdq{i}")
                for i in range(2)
            ]
            for t in bdqs:
                nc.vector.memset(t, 0.0)

            xts = {}
            kfs = {}
            mns = {}
            pending_o = []

            def oproj_flush():
                while pending_o:
                    og, oao = pending_o.pop(0)
                    for tc2 in range(2):
                        ps_o = psp.tile([128, 512], F32, tag="ps", name="ps_o")
                        for ec in range(4):
                            nc.tensor.matmul(
                                ps_o,
                                oao[:, ec, tc2 * 128 : (tc2 + 1) * 128],
                                wo_t[:, ec, :],
                                start=(ec == 0), stop=(ec == 3),
                            )
                        out_s = osp.tile([128, 512], F32, tag="outs", name="out_s")
                        nc.scalar.activation(
                            out_s, ps_o, AF.Copy,
                            scale=pm_t[:, og * 2 + tc2 : og * 2 + tc2 + 1],
                        )
                        q0 = og * GB * NQ
                        nc.sync.dma_start(
                            out[q0 + tc2 * 128 : q0 + (tc2 + 1) * 128, :], out_s
                        )

            def load_xt(g):
                xt = xtp.tile([128, 4, GB * NE], BF16, tag="xt", name=f"xt{g}")
                t0 = g * GB * NE
                for h in range(2):
                    nc.sync.dma_start(
                        xt[:, :, h * 512 : (h + 1) * 512],
                        ent_r[:, :, t0 + h * 512 : t0 + (h + 1) * 512],
                    )
                xts[g] = xt

            def load_mn(g):
                mn = mnp.tile([128, 8, 64], BF16, tag="mn", name=f"mn{g}")
                nc.sync.dma_start(
                    mn, msk.rearrange("p (g x) -> p g x", g=NG)[:, g, :]
                )
                mns[g] = mn

            def kproj(g):
                """K projection for group g -> kf[g] (bf16)."""
                xt = xts[g]
                kf = kfp.tile([128, 4, GB * NE], BF16, tag="kf", name=f"kf{g}")
                for ec in range(4):
                    for fg in range(2):
                        ps_k = psp.tile([128, 512], F32, tag="ps", name="ps_k")
                        for dc in range(4):
                            nc.tensor.matmul(
                                ps_k,
                                wk_t[:, dc, ec * 128 : (ec + 1) * 128],
                                xt[:, dc, fg * 512 : (fg + 1) * 512],
                                start=(dc == 0), stop=(dc == 3),
                            )
                        nc.scalar.copy(kf[:, ec, fg * 512 : (fg + 1) * 512], ps_k)
                kfs[g] = kf

            def qproj(g):
                """Q projection for group g -> block-diagonal bdq[g%2]."""
                xt = xts[g]
                bdq = bdqs[g % 2]
                xq_view = xt.rearrange("p dc (b t) -> p dc b t", b=GB)
                for eh in range(2):
                    ps_q = psp.tile([128, 2, 256], F32, tag="ps", name="ps_q")
                    for ei in range(2):
                        ec = eh * 2 + ei
                        for dc in range(4):
                            nc.tensor.matmul(
                                ps_q[:, ei, :],
                                wq_t[:, dc, ec * 128 : (ec + 1) * 128],
                                xq_view[:, dc, :, 0:NQ],
                                start=(dc == 0), stop=(dc == 3),
                            )
                    ps_qv = ps_q.rearrange("p c (g2 x) -> p c g2 x", g2=8)
                    cs = slice(eh * 2, eh * 2 + 2)
                    nc.scalar.copy(bdq[0:64, cs, :, 0:32], ps_qv[0:64])
                    nc.scalar.copy(bdq[64:128, cs, :, 32:64], ps_qv[64:128])

            # ---- prologue: group 0's K/Q + first mask ----
            load_xt(0)
            kproj(0)
            qproj(0)
            load_mn(0)

            for g in range(NG):
                xt = xts.pop(g)
                kf = kfs.pop(g)
                mn = mns.pop(g)
                bdq = bdqs[g % 2]

                # ---- logits + mask + exp -> bde ----
                bde = bdep.tile([128, 4, 8, 64], BF16, tag="bde", name=f"bde{g}")
                for c in range(4):
                    ps_l = pslp.tile([128, 8, 64], F32, tag="psl", name="ps_l")
                    for g2 in range(8):
                        nc.tensor.matmul(
                            ps_l[:, g2, :],
                            kf[:, c, g2 * 128 : (g2 + 1) * 128],
                            bdq[:, c, g2, :],
                            start=True, stop=True,
                        )
                    nc.scalar.activation(
                        bde[:, c, :, :], ps_l, AF.Exp, scale=1.0 / np.sqrt(HD)
                    )
                    # zero masked + cross-batch junk cells on the idle Pool engine
                    nc.gpsimd.tensor_tensor(
                        bde[:, c, :, :], bde[:, c, :, :], mn, ALU.mult
                    )

                if debug and g == 0:
                    nc.sync.dma_start(dbg["dxt"].rearrange("a b -> a b"), xt.rearrange("p a b -> p (a b)"))
                    nc.sync.dma_start(dbg["dkf"].rearrange("a b -> a b"), kf.rearrange("p a b -> p (a b)"))
                    nc.sync.dma_start(dbg["dbdq"].rearrange("a b -> a b"), bdq.rearrange("p a b c -> p (a b c)"))
                    nc.sync.dma_start(dbg["dbde"].rearrange("a b -> a b"), bde.rearrange("p a b c -> p (a b c)"))

                # ---- previous group's output projection (covers extract) ----
                oproj_flush()

                # ---- V projection (overlaps exp on ACT) ----
                vt = vtp.tile([128, 8, 512], BF16, tag="vt", name=f"vt{g}")
                for t8 in range(8):
                    ps_v = psp.tile([128, 512], F32, tag="ps", name="ps_v")
                    for dc in range(4):
                        nc.tensor.matmul(
                            ps_v,
                            xt[:, dc, t8 * 128 : (t8 + 1) * 128],
                            wv_t[:, dc, :],
                            start=(dc == 0), stop=(dc == 3),
                        )
                    if t8 < 4:
                        nc.scalar.copy(vt[:, t8, :], ps_v)
                    else:
                        nc.vector.tensor_copy(vt[:, t8, :], ps_v)

                # ---- softmax denominators (replicated across partitions) ----
                rs = rsp.tile([128, 4, 512], F32, tag="rs", name=f"rs{g}")
                for c in range(4):
                    ps_s = pslp.tile([128, 512], F32, tag="psl", name="ps_s")
                    nc.tensor.matmul(
                        ps_s,
                        ones_t,
                        bde[:, c, :, :].rearrange("p a b -> p (a b)"),
                        start=True, stop=True,
                    )
                    nc.vector.reciprocal_approx_fast(out=rs[:, c, :], in_=ps_s)

                if debug and g == 0:
                    nc.sync.dma_start(dbg["drs"].rearrange("a b -> a b"), rs.rearrange("p a b -> p (a b)"))
                    nc.sync.dma_start(dbg["dvt"].rearrange("a b -> a b"), vt.rearrange("p a b -> p (a b)"))

                # ---- attnv interleaved with next group's K so the small
                # stationary loads hide under 512-wide K matmuls ----
                if g + 1 < NG:
                    load_xt(g + 1)
                    load_mn(g + 1)
                    xt_n = xts[g + 1]
                    kf_n = kfp.tile([128, 4, GB * NE], BF16, tag="kf", name=f"kf{g+1}")
                    kfs[g + 1] = kf_n
                else:
                    xt_n = kf_n = None

                ao = aop.tile([128, 4, 256], BF16, tag="ao", name=f"ao{g}")
                ao_v = ao.rearrange("p c (h2 gi x) -> p c h2 gi x", h2=2, gi=4)
                rs_v = rs.rearrange("p c (g2 h2 x) -> p c g2 h2 x", g2=8, h2=2)
                avs = []
                for half in range(2):
                    av = avp.tile([128, 4, 4, 64], F32, tag="av", name="ps_av")
                    avs.append(av)
                # emit pairs: one K matmul, one attnv matmul
                kmms = []
                if kf_n is not None:
                    for ec in range(4):
                        for fg in range(2):
                            kmms.append((ec, fg))
                amms = [(half, gi, c) for half in range(2) for gi in range(4)
                        for c in range(4)]
                ki = 0
                kps = None
                for ai, (half, gi, c) in enumerate(amms):
                    # one dc-accumulation step of K per attnv matmul
                    if kmms:
                        kidx = ai // 4
                        dc = ai % 4
                        if kidx < len(kmms):
                            ec, fg = kmms[kidx]
                            if dc == 0:
                                kps = psp.tile([128, 512], F32, tag="ps", name="ps_k")
                            nc.tensor.matmul(
                                kps,
                                wk_t[:, dc, ec * 128 : (ec + 1) * 128],
                                xt_n[:, dc, fg * 512 : (fg + 1) * 512],
                                start=(dc == 0), stop=(dc == 3),
                            )
                            if dc == 3:
                                dst = kf_n[:, ec, fg * 512 : (fg + 1) * 512]
                                if ec < 3:
                                    nc.scalar.copy(dst, kps)
                                else:
                                    nc.vector.tensor_copy(dst, kps)
                    g2 = half * 4 + gi
                    nc.tensor.matmul(
                        avs[half][:, gi, c, :],
                        vt[:, g2, c * 128 : (c + 1) * 128],
                        bde[:, c, g2, :],
                        start=True, stop=True,
                    )
                    if ai % 16 == 15:
                        half_done = ai // 16
                        av_v = avs[half_done].rearrange("p gi c x -> p c gi x")
                        for P in range(2):
                            psl = slice(P * 64, (P + 1) * 64)
                            nc.vector.tensor_tensor(
                                ao_v[psl, :, half_done, :, :],
                                av_v[psl, :, :, P * 32 : (P + 1) * 32],
                                rs_v[psl, :, half_done * 4 : (half_done + 1) * 4, P, :],
                                ALU.mult,
                            )
                if g + 1 < NG:
                    qproj(g + 1)

                if debug and g == 0:
                    nc.sync.dma_start(dbg["dao"].rearrange("a b -> a b"), ao.rearrange("p a b -> p (a b)"))

                pending_o.append((g, ao))

            oproj_flush()

    nc.finalize()
    return nc


_NC_CACHE = None
RUN_KWARGS = {}
LAST_RESULT = None


def _get_nc():
    global _NC_CACHE
    if _NC_CACHE is None:
        _NC_CACHE = build_nc()
    return _NC_CACHE


def _bf16(x):
    return np.ascontiguousarray(x.astype(ml_dtypes.bfloat16))


def kernel(entities, pre_mask, post_mask, W_in, W_out, b_out):
    entities = np.asarray(entities, dtype=np.float32)
    pre_mask = np.asarray(pre_mask)
    post_mask = np.asarray(post_mask)
    W_in = np.asarray(W_in, dtype=np.float32)
    W_out = np.asarray(W_out, dtype=np.float32)
    b_out = np.asarray(b_out, dtype=np.float32)

    wqT = _bf16(W_in[0:512].T)
    wkT = _bf16(W_in[512:1024].T)
    wvT = _bf16(W_in[1024:1536].T)
    woT = _bf16(W_out.T)

    bp_idx = np.arange(2).reshape(2, 1, 1, 1, 1, 1, 1)
    B_idx = np.arange(2).reshape(1, 1, 1, 1, 1, 2, 1)

    in_maps = []
    for i in range(NCORES):
        bsl = slice(i * BPC, (i + 1) * BPC)
        ent_i = _bf16(entities[bsl].reshape(NTOK, D).T)
        # mneg[(bp,j), (g,g2,P,B,q)]: -1e30 where cross-batch or pre-masked
        pm_i = pre_mask[bsl, :NQ, :]                       # (256, 16, 64)
        pm_r = pm_i.reshape(NG, 8, 2, NQ, NE)              # (g, g2, B, q, j)
        pmx = pm_r.transpose(4, 0, 1, 2, 3)                # (j, g, g2, B, q)
        cond = bp_idx != B_idx                             # (2,1,1,1,1,2,1)
        cond = cond | pmx[None, :, :, :, None, :, :]       # (2,j,g,g2,P,B,q)
        cond = np.broadcast_to(cond, (2, NE, NG, 8, 2, 2, NQ))
        msk_i = _bf16(np.where(cond, 0.0, 1.0).reshape(128, NG * 512))
        pmt_i = np.ascontiguousarray(
            (1.0 - post_mask[bsl].astype(np.float32)).reshape(NQT)
        )
        in_maps.append(
            {
                "ent": ent_i,
                "msk": msk_i,
                "pmt": pmt_i,
                "wqT": wqT,
                "wkT": wkT,
                "wvT": wvT,
                "woT": woT,
            }
        )

    nc = _get_nc()
    res = bass_utils.run_bass_kernel_spmd(
        nc, in_maps, list(range(NCORES)), **RUN_KWARGS
    )
    global LAST_RESULT
    LAST_RESULT = res
    outs = [res.results[i]["out"].reshape(BPC, NQ, 512) for i in range(NCORES)]
    full = np.concatenate(outs, axis=0)
    if b_out.any():
        full = full + b_out[None, None, :]
        full = np.where(post_mask[:, :, None], 0.0, full)
    return full.astype(np.float32)


# revision 16
# speedup vs baseline: 1.0512x; 1.0373x over previous
"""EntityAttentionLayer Trainium2 kernel (v2, all-bf16 datapath).

Data-parallel over batch across 8 NeuronCores (256 batches/core).
Per core, per G-group of 16 batches (1024 entity tokens, 256 query tokens),
software-pipelined so group g's attention runs while group g+1's K/Q
projections keep the PE busy:

  xt:    entities, feature-on-partition, bf16        [128, 4dc, 1024]
  kf:    K feature-major GEMM -> bf16                [128, 4c, 1024]
  vt:    V token-major GEMM -> bf16                  [128, 8t, 512]
  bdq:   Q gathered from xt via strided moving AP, written block-diagonal
         over head-parity                            [128, 4c, 8g2, 64]
  logits: per (c, g2): kf[128,128] stationary (2 batches) x bdq cols
         -> psum [128 tok, 8g2, 64]; junk cross-batch cells + pre-mask
         handled by ONE identity-stationary matmul adding mneg (-1e30)
  exp:   ACT -> bde bf16 (masked cells exp -> 0)     [128, 4c, 8g2, 64]
  sums:  all-ones [128,128] stationary matmul -> denominators replicated
         across all 128 partitions; reciprocal_approx_fast -> rs f32
  attnv: vt stationary x bde moving -> psum with junk parity blocks;
         diagonal blocks extracted * rs (fused normalize) -> ao bf16
  oproj: ao stationary x woT moving; post-mask applied via ACT Copy with
         per-partition scale; DMA out f32
"""

import os
import sys

import numpy as np

sys.path.insert(0, "/opt/trn_rl_repo")

import concourse.bass as bass
import concourse.mybir as mybir
import concourse.tile as tile
from concourse import bacc, bass_utils

import ml_dtypes

F32 = mybir.dt.float32
BF16 = mybir.dt.bfloat16
AF = mybir.ActivationFunctionType
ALU = mybir.AluOpType

BS, NE, NQ = 2048, 64, 16
D = 512
H = 8
HD = 64
NCORES = 8
BPC = BS // NCORES          # 256 batches per core
GB = 16                     # batches per G-group
NG = BPC // GB              # 16 groups
NTOK = BPC * NE             # 16384 entity tokens per core
NQT = BPC * NQ              # 4096 query tokens per core


def build_nc(debug=False):
    nc = bacc.Bacc()
    dbg = {}
    if debug:
        for nm, shp, dt in [
            ("dxt", [128, 4096], BF16), ("dkf", [128, 4096], BF16),
            ("dbdq", [128, 2048], BF16), ("dbde", [128, 2048], BF16),
            ("dvt", [128, 4096], BF16), ("drs", [128, 2048], F32),
            ("dao", [128, 1024], BF16),
        ]:
            dbg[nm] = nc.declare_dram_parameter(nm, shp, dt, isOutput=True)

    ent = nc.declare_dram_parameter("ent", [D, NTOK], BF16, isOutput=False)
    msk = nc.declare_dram_parameter("msk", [128, NG * 512], BF16, isOutput=False)
    pmt = nc.declare_dram_parameter("pmt", [NQT], F32, isOutput=False)
    wqT = nc.declare_dram_parameter("wqT", [D, 512], BF16, isOutput=False)
    wkT = nc.declare_dram_parameter("wkT", [D, 512], BF16, isOutput=False)
    wvT = nc.declare_dram_parameter("wvT", [D, 512], BF16, isOutput=False)
    woT = nc.declare_dram_parameter("woT", [512, 512], BF16, isOutput=False)
    out = nc.declare_dram_parameter("out", [NQT, 512], F32, isOutput=True)

    ent_r = ent.rearrange("(dc p) n -> p dc n", p=128)   # [128, 4, 16384]
    pmt_r = pmt.rearrange("(o p) -> p o", p=128)         # [128, 32]

    with tile.TileContext(nc) as tc:
        with (
            tc.tile_pool(name="wpool", bufs=1) as wpool,
            tc.tile_pool(name="xtp", bufs=2) as xtp,
            tc.tile_pool(name="kfp", bufs=2) as kfp,
            tc.tile_pool(name="vtp", bufs=2) as vtp,
            tc.tile_pool(name="persist", bufs=1) as persist,
            tc.tile_pool(name="bdep", bufs=2) as bdep,
            tc.tile_pool(name="mnp", bufs=2) as mnp,
            tc.tile_pool(name="rsp", bufs=2) as rsp,
            tc.tile_pool(name="aop", bufs=2) as aop,
            tc.tile_pool(name="osp", bufs=2) as osp,
            tc.tile_pool(name="psp", bufs=2, space="PSUM") as psp,
            tc.tile_pool(name="pslp", bufs=2, space="PSUM") as pslp,
            tc.tile_pool(name="avp", bufs=2, space="PSUM") as avp,
        ):
            # ---- constants / weights (loaded once) ----
            wq_t = wpool.tile([128, 4, 512], BF16, tag="wq")
            wk_t = wpool.tile([128, 4, 512], BF16, tag="wk")
            wv_t = wpool.tile([128, 4, 512], BF16, tag="wv")
            wo_t = wpool.tile([128, 4, 512], BF16, tag="wo")
            nc.sync.dma_start(wk_t, wkT.rearrange("(dc p) e -> p dc e", p=128))
            nc.sync.dma_start(wq_t, wqT.rearrange("(dc p) e -> p dc e", p=128))
            nc.sync.dma_start(wv_t, wvT.rearrange("(dc p) e -> p dc e", p=128))
            nc.sync.dma_start(wo_t, woT.rearrange("(ec p) o -> p ec o", p=128))
            pm_t = wpool.tile([128, 32], F32, tag="pm")
            nc.sync.dma_start(pm_t, pmt_r)
            ones_t = wpool.tile([128, 128], BF16, tag="ones")
            nc.vector.memset(ones_t, 1.0)

            # persistent block-diagonal Q (off-parity blocks stay zero)
            bdqs = [
                persist.tile([128, 4, 8, 64], BF16, tag=f"bdq{i}", name=f"bdq{i}")
                for i in range(2)
            ]
            for t in bdqs:
                nc.vector.memset(t, 0.0)

            xts = {}
            kfs = {}
            mns = {}
            pending_o = []

            def oproj_flush():
                while pending_o:
                    og, oao = pending_o.pop(0)
                    for tc2 in range(2):
                        ps_o = psp.tile([128, 512], F32, tag="ps", name="ps_o")
                        for ec in range(4):
                            nc.tensor.matmul(
                                ps_o,
                                oao[:, ec, tc2 * 128 : (tc2 + 1) * 128],
                                wo_t[:, ec, :],
                                start=(ec == 0), stop=(ec == 3),
                            )
                        out_s = osp.tile([128, 512], F32, tag="outs", name="out_s")
                        nc.scalar.activation(
                            out_s, ps_o, AF.Copy,
                            scale=pm_t[:, og * 2 + tc2 : og * 2 + tc2 + 1],
                        )
                        q0 = og * GB * NQ
                        nc.sync.dma_start(
                            out[q0 + tc2 * 128 : q0 + (tc2 + 1) * 128, :], out_s
                        )

            def load_xt(g):
                xt = xtp.tile([128, 4, GB * NE], BF16, tag="xt", name=f"xt{g}")
                t0 = g * GB * NE
                for h in range(2):
                    nc.sync.dma_start(
                        xt[:, :, h * 512 : (h + 1) * 512],
                        ent_r[:, :, t0 + h * 512 : t0 + (h + 1) * 512],
                    )
                xts[g] = xt

            def load_mn(g):
                mn = mnp.tile([128, 8, 64], BF16, tag="mn", name=f"mn{g}")
                nc.sync.dma_start(
                    mn, msk.rearrange("p (g x) -> p g x", g=NG)[:, g, :]
                )
                mns[g] = mn

            def kproj(g):
                """K projection for group g -> kf[g] (bf16)."""
                xt = xts[g]
                kf = kfp.tile([128, 4, GB * NE], BF16, tag="kf", name=f"kf{g}")
                for ec in range(4):
                    for fg in range(2):
                        ps_k = psp.tile([128, 512], F32, tag="ps", name="ps_k")
                        for dc in range(4):
                            nc.tensor.matmul(
                                ps_k,
                                wk_t[:, dc, ec * 128 : (ec + 1) * 128],
                                xt[:, dc, fg * 512 : (fg + 1) * 512],
                                start=(dc == 0), stop=(dc == 3),
                            )
                        nc.scalar.copy(kf[:, ec, fg * 512 : (fg + 1) * 512], ps_k)
                kfs[g] = kf

            def qproj(g):
                """Q projection for group g -> block-diagonal bdq[g%2]."""
                xt = xts[g]
                bdq = bdqs[g % 2]
                xq_view = xt.rearrange("p dc (b t) -> p dc b t", b=GB)
                for eh in range(2):
                    ps_q = psp.tile([128, 2, 256], F32, tag="ps", name="ps_q")
                    for ei in range(2):
                        ec = eh * 2 + ei
                        for dc in range(4):
                            nc.tensor.matmul(
                                ps_q[:, ei, :],
                                wq_t[:, dc, ec * 128 : (ec + 1) * 128],
                                xq_view[:, dc, :, 0:NQ],
                                start=(dc == 0), stop=(dc == 3),
                            )
                    ps_qv = ps_q.rearrange("p c (g2 x) -> p c g2 x", g2=8)
                    cs = slice(eh * 2, eh * 2 + 2)
                    nc.scalar.copy(bdq[0:64, cs, :, 0:32], ps_qv[0:64])
                    nc.scalar.copy(bdq[64:128, cs, :, 32:64], ps_qv[64:128])

            # ---- prologue: group 0's K/Q + first mask ----
            load_xt(0)
            load_mn(0)
            kproj(0)
            qproj(0)

            for g in range(NG):
                xt = xts.pop(g)
                kf = kfs.pop(g)
                mn = mns.pop(g)
                bdq = bdqs[g % 2]

                # ---- logits + mask + exp -> bde ----
                bde = bdep.tile([128, 4, 8, 64], BF16, tag="bde", name=f"bde{g}")
                for c in range(4):
                    ps_l = pslp.tile([128, 8, 64], F32, tag="psl", name="ps_l")
                    for g2 in range(8):
                        nc.tensor.matmul(
                            ps_l[:, g2, :],
                            kf[:, c, g2 * 128 : (g2 + 1) * 128],
                            bdq[:, c, g2, :],
                            start=True, stop=True,
                        )
                    nc.scalar.activation(
                        bde[:, c, :, :], ps_l, AF.Exp, scale=1.0 / np.sqrt(HD)
                    )
                    # zero masked + cross-batch junk cells on the idle Pool engine
                    nc.gpsimd.tensor_tensor(
                        bde[:, c, :, :], bde[:, c, :, :], mn, ALU.mult
                    )

                if debug and g == 0:
                    nc.sync.dma_start(dbg["dxt"].rearrange("a b -> a b"), xt.rearrange("p a b -> p (a b)"))
                    nc.sync.dma_start(dbg["dkf"].rearrange("a b -> a b"), kf.rearrange("p a b -> p (a b)"))
                    nc.sync.dma_start(dbg["dbdq"].rearrange("a b -> a b"), bdq.rearrange("p a b c -> p (a b c)"))
                    nc.sync.dma_start(dbg["dbde"].rearrange("a b -> a b"), bde.rearrange("p a b c -> p (a b c)"))

                # ---- previous group's output projection (covers extract) ----
                oproj_flush()

                # ---- V projection (overlaps exp on ACT) ----
                vt = vtp.tile([128, 8, 512], BF16, tag="vt", name=f"vt{g}")
                for t8 in range(8):
                    ps_v = psp.tile([128, 512], F32, tag="ps", name="ps_v")
                    for dc in range(4):
                        nc.tensor.matmul(
                            ps_v,
                            xt[:, dc, t8 * 128 : (t8 + 1) * 128],
                            wv_t[:, dc, :],
                            start=(dc == 0), stop=(dc == 3),
                        )
                    if t8 < 4:
                        nc.scalar.copy(vt[:, t8, :], ps_v)
                    else:
                        nc.vector.tensor_copy(vt[:, t8, :], ps_v)

                # ---- softmax denominators (replicated across partitions) ----
                rs = rsp.tile([128, 4, 512], F32, tag="rs", name=f"rs{g}")
                for c in range(4):
                    ps_s = pslp.tile([128, 512], F32, tag="psl", name="ps_s")
                    nc.tensor.matmul(
                        ps_s,
                        ones_t,
                        bde[:, c, :, :].rearrange("p a b -> p (a b)"),
                        start=True, stop=True,
                    )
                    nc.vector.reciprocal_approx_fast(out=rs[:, c, :], in_=ps_s)

                if debug and g == 0:
                    nc.sync.dma_start(dbg["drs"].rearrange("a b -> a b"), rs.rearrange("p a b -> p (a b)"))
                    nc.sync.dma_start(dbg["dvt"].rearrange("a b -> a b"), vt.rearrange("p a b -> p (a b)"))

                # ---- next group's K/Q keep the PE busy while DVE/ACT drain ----
                if g + 1 < NG:
                    load_xt(g + 1)
                    load_mn(g + 1)
                    kproj(g + 1)
                    qproj(g + 1)

                # ---- attn @ V with junk parity blocks; extract diag * rs ----
                ao = aop.tile([128, 4, 256], BF16, tag="ao", name=f"ao{g}")
                ao_v = ao.rearrange("p c (h2 gi x) -> p c h2 gi x", h2=2, gi=4)
                rs_v = rs.rearrange("p c (g2 h2 x) -> p c g2 h2 x", g2=8, h2=2)
                for half in range(2):
                    av = avp.tile([128, 4, 4, 64], F32, tag="av", name="ps_av")
                    for gi in range(4):
                        g2 = half * 4 + gi
                        for c in range(4):
                            nc.tensor.matmul(
                                av[:, gi, c, :],
                                vt[:, g2, c * 128 : (c + 1) * 128],
                                bde[:, c, g2, :],
                                start=True, stop=True,
                            )
                    av_v = av.rearrange("p gi c x -> p c gi x")
                    for P in range(2):
                        psl = slice(P * 64, (P + 1) * 64)
                        nc.vector.tensor_tensor(
                            ao_v[psl, :, half, :, :],
                            av_v[psl, :, :, P * 32 : (P + 1) * 32],
                            rs_v[psl, :, half * 4 : (half + 1) * 4, P, :],
                            ALU.mult,
                        )

                if debug and g == 0:
                    nc.sync.dma_start(dbg["dao"].rearrange("a b -> a b"), ao.rearrange("p a b -> p (a b)"))

                pending_o.append((g, ao))

            oproj_flush()

    nc.finalize()
    return nc


_NC_CACHE = None
RUN_KWARGS = {}
LAST_RESULT = None


def _get_nc():
    global _NC_CACHE
    if _NC_CACHE is None:
        _NC_CACHE = build_nc()
    return _NC_CACHE


def _bf16(x):
    return np.ascontiguousarray(x.astype(ml_dtypes.bfloat16))


def kernel(entities, pre_mask, post_mask, W_in, W_out, b_out):
    entities = np.asarray(entities, dtype=np.float32)
    pre_mask = np.asarray(pre_mask)
    post_mask = np.asarray(post_mask)
    W_in = np.asarray(W_in, dtype=np.float32)
    W_out = np.asarray(W_out, dtype=np.float32)
    b_out = np.asarray(b_out, dtype=np.float32)

    wqT = _bf16(W_in[0:512].T)
    wkT = _bf16(W_in[512:1024].T)
    wvT = _bf16(W_in[1024:1536].T)
    woT = _bf16(W_out.T)

    bp_idx = np.arange(2).reshape(2, 1, 1, 1, 1, 1, 1)
    B_idx = np.arange(2).reshape(1, 1, 1, 1, 1, 2, 1)

    in_maps = []
    for i in range(NCORES):
        bsl = slice(i * BPC, (i + 1) * BPC)
        ent_i = _bf16(entities[bsl].reshape(NTOK, D).T)
        # mneg[(bp,j), (g,g2,P,B,q)]: -1e30 where cross-batch or pre-masked
        pm_i = pre_mask[bsl, :NQ, :]                       # (256, 16, 64)
        pm_r = pm_i.reshape(NG, 8, 2, NQ, NE)              # (g, g2, B, q, j)
        pmx = pm_r.transpose(4, 0, 1, 2, 3)                # (j, g, g2, B, q)
        cond = bp_idx != B_idx                             # (2,1,1,1,1,2,1)
        cond = cond | pmx[None, :, :, :, None, :, :]       # (2,j,g,g2,P,B,q)
        cond = np.broadcast_to(cond, (2, NE, NG, 8, 2, 2, NQ))
        msk_i = _bf16(np.where(cond, 0.0, 1.0).reshape(128, NG * 512))
        pmt_i = np.ascontiguousarray(
            (1.0 - post_mask[bsl].astype(np.float32)).reshape(NQT)
        )
        in_maps.append(
            {
                "ent": ent_i,
                "msk": msk_i,
                "pmt": pmt_i,
                "wqT": wqT,
                "wkT": wkT,
                "wvT": wvT,
                "woT": woT,
            }
        )

    nc = _get_nc()
    res = bass_utils.run_bass_kernel_spmd(
        nc, in_maps, list(range(NCORES)), **RUN_KWARGS
    )
    global LAST_RESULT
    LAST_RESULT = res
    outs = [res.results[i]["out"].reshape(BPC, NQ, 512) for i in range(NCORES)]
    full = np.concatenate(outs, axis=0)
    if b_out.any():
        full = full + b_out[None, None, :]
        full = np.where(post_mask[:, :, None], 0.0, full)
    return full.astype(np.float32)


# revision 21
# speedup vs baseline: 1.0718x; 1.0196x over previous
"""EntityAttentionLayer Trainium2 kernel (v2, all-bf16 datapath).

Data-parallel over batch across 8 NeuronCores (256 batches/core).
Per core, per G-group of 16 batches (1024 entity tokens, 256 query tokens),
software-pipelined so group g's attention runs while group g+1's K/Q
projections keep the PE busy:

  xt:    entities, feature-on-partition, bf16        [128, 4dc, 1024]
  kf:    K feature-major GEMM -> bf16                [128, 4c, 1024]
  vt:    V token-major GEMM -> bf16                  [128, 8t, 512]
  bdq:   Q gathered from xt via strided moving AP, written block-diagonal
         over head-parity                            [128, 4c, 8g2, 64]
  logits: per (c, g2): kf[128,128] stationary (2 batches) x bdq cols
         -> psum [128 tok, 8g2, 64]; junk cross-batch cells + pre-mask
         handled by ONE identity-stationary matmul adding mneg (-1e30)
  exp:   ACT -> bde bf16 (masked cells exp -> 0)     [128, 4c, 8g2, 64]
  sums:  all-ones [128,128] stationary matmul -> denominators replicated
         across all 128 partitions; reciprocal_approx_fast -> rs f32
  attnv: vt stationary x bde moving -> psum with junk parity blocks;
         diagonal blocks extracted * rs (fused normalize) -> ao bf16
  oproj: ao stationary x woT moving; post-mask applied via ACT Copy with
         per-partition scale; DMA out f32
"""

import os
import sys

import numpy as np

sys.path.insert(0, "/opt/trn_rl_repo")

import concourse.bass as bass
import concourse.mybir as mybir
import concourse.tile as tile
from concourse import bacc, bass_utils

import ml_dtypes

F32 = mybir.dt.float32
BF16 = mybir.dt.bfloat16
AF = mybir.ActivationFunctionType
ALU = mybir.AluOpType

BS, NE, NQ = 2048, 64, 16
D = 512
H = 8
HD = 64
NCORES = 8
BPC = BS // NCORES          # 256 batches per core
GB = 16                     # batches per G-group
NG = BPC // GB              # 16 groups
NTOK = BPC * NE             # 16384 entity tokens per core
NQT = BPC * NQ              # 4096 query tokens per core


def build_nc(debug=False):
    nc = bacc.Bacc()
    dbg = {}
    if debug:
        for nm, shp, dt in [
            ("dxt", [128, 4096], BF16), ("dkf", [128, 4096], BF16),
            ("dbdq", [128, 2048], BF16), ("dbde", [128, 2048], BF16),
            ("dvt", [128, 4096], BF16), ("drs", [128, 2048], F32),
            ("dao", [128, 1024], BF16),
        ]:
            dbg[nm] = nc.declare_dram_parameter(nm, shp, dt, isOutput=True)

    ent = nc.declare_dram_parameter("ent", [D, NTOK], BF16, isOutput=False)
    msk = nc.declare_dram_parameter("msk", [128, NG * 512], BF16, isOutput=False)
    pmt = nc.declare_dram_parameter("pmt", [NQT], F32, isOutput=False)
    wqT = nc.declare_dram_parameter("wqT", [D, 512], BF16, isOutput=False)
    wkT = nc.declare_dram_parameter("wkT", [D, 512], BF16, isOutput=False)
    wvT = nc.declare_dram_parameter("wvT", [D, 512], BF16, isOutput=False)
    woT = nc.declare_dram_parameter("woT", [512, 512], BF16, isOutput=False)
    out = nc.declare_dram_parameter("out", [NQT, 512], F32, isOutput=True)

    ent_r = ent.rearrange("(dc p) n -> p dc n", p=128)   # [128, 4, 16384]
    pmt_r = pmt.rearrange("(o p) -> p o", p=128)         # [128, 32]

    with tile.TileContext(nc) as tc:
        with (
            tc.tile_pool(name="wpool", bufs=1) as wpool,
            tc.tile_pool(name="xtp", bufs=2) as xtp,
            tc.tile_pool(name="kfp", bufs=2) as kfp,
            tc.tile_pool(name="vtp", bufs=2) as vtp,
            tc.tile_pool(name="persist", bufs=1) as persist,
            tc.tile_pool(name="bdep", bufs=2) as bdep,
            tc.tile_pool(name="mnp", bufs=2) as mnp,
            tc.tile_pool(name="rsp", bufs=2) as rsp,
            tc.tile_pool(name="aop", bufs=2) as aop,
            tc.tile_pool(name="osp", bufs=2) as osp,
            tc.tile_pool(name="psp", bufs=2, space="PSUM") as psp,
            tc.tile_pool(name="pslp", bufs=2, space="PSUM") as pslp,
            tc.tile_pool(name="avp", bufs=2, space="PSUM") as avp,
        ):
            # ---- constants / weights (loaded once) ----
            wq_t = wpool.tile([128, 4, 512], BF16, tag="wq")
            wk_t = wpool.tile([128, 4, 512], BF16, tag="wk")
            wv_t = wpool.tile([128, 4, 512], BF16, tag="wv")
            wo_t = wpool.tile([128, 4, 512], BF16, tag="wo")
            nc.sync.dma_start(wk_t, wkT.rearrange("(dc p) e -> p dc e", p=128))
            pm_t = wpool.tile([128, 32], F32, tag="pm")
            ones_t = wpool.tile([128, 128], BF16, tag="ones")
            nc.vector.memset(ones_t, 1.0)

            # persistent block-diagonal Q (off-parity blocks stay zero)
            bdqs = [
                persist.tile([128, 4, 8, 64], BF16, tag=f"bdq{i}", name=f"bdq{i}")
                for i in range(2)
            ]
            for t in bdqs:
                nc.vector.memset(t, 0.0)

            xts = {}
            kfs = {}
            mns = {}
            pending_o = []

            def oproj_flush():
                while pending_o:
                    og, oao = pending_o.pop(0)
                    for tc2 in range(2):
                        ps_o = psp.tile([128, 512], F32, tag="ps", name="ps_o")
                        for ec in range(4):
                            nc.tensor.matmul(
                                ps_o,
                                oao[:, ec, tc2 * 128 : (tc2 + 1) * 128],
                                wo_t[:, ec, :],
                                start=(ec == 0), stop=(ec == 3),
                            )
                        out_s = osp.tile([128, 512], F32, tag="outs", name="out_s")
                        nc.scalar.activation(
                            out_s, ps_o, AF.Copy,
                            scale=pm_t[:, og * 2 + tc2 : og * 2 + tc2 + 1],
                        )
                        q0 = og * GB * NQ
                        nc.sync.dma_start(
                            out[q0 + tc2 * 128 : q0 + (tc2 + 1) * 128, :], out_s
                        )

            def load_xt(g):
                xt = xtp.tile([128, 4, GB * NE], BF16, tag="xt", name=f"xt{g}")
                t0 = g * GB * NE
                for h in range(2):
                    nc.sync.dma_start(
                        xt[:, :, h * 512 : (h + 1) * 512],
                        ent_r[:, :, t0 + h * 512 : t0 + (h + 1) * 512],
                    )
                xts[g] = xt

            def load_mn(g):
                mn = mnp.tile([128, 8, 64], BF16, tag="mn", name=f"mn{g}")
                nc.sync.dma_start(
                    mn, msk.rearrange("p (g x) -> p g x", g=NG)[:, g, :]
                )
                mns[g] = mn

            def kproj(g):
                """K projection for group g -> kf[g] (bf16)."""
                xt = xts[g]
                kf = kfp.tile([128, 4, GB * NE], BF16, tag="kf", name=f"kf{g}")
                for ec in range(4):
                    for fg in range(2):
                        ps_k = psp.tile([128, 512], F32, tag="ps", name="ps_k")
                        for dc in range(4):
                            nc.tensor.matmul(
                                ps_k,
                                wk_t[:, dc, ec * 128 : (ec + 1) * 128],
                                xt[:, dc, fg * 512 : (fg + 1) * 512],
                                start=(dc == 0), stop=(dc == 3),
                            )
                        nc.scalar.copy(kf[:, ec, fg * 512 : (fg + 1) * 512], ps_k)
                kfs[g] = kf

            def qproj(g):
                """Q projection for group g -> block-diagonal bdq[g%2]."""
                xt = xts[g]
                bdq = bdqs[g % 2]
                xq_view = xt.rearrange("p dc (b t) -> p dc b t", b=GB)
                for eh in range(2):
                    ps_q = psp.tile([128, 2, 256], F32, tag="ps", name="ps_q")
                    for ei in range(2):
                        ec = eh * 2 + ei
                        for dc in range(4):
                            nc.tensor.matmul(
                                ps_q[:, ei, :],
                                wq_t[:, dc, ec * 128 : (ec + 1) * 128],
                                xq_view[:, dc, :, 0:NQ],
                                start=(dc == 0), stop=(dc == 3),
                            )
                    ps_qv = ps_q.rearrange("p c (g2 x) -> p c g2 x", g2=8)
                    cs = slice(eh * 2, eh * 2 + 2)
                    nc.scalar.copy(bdq[0:64, cs, :, 0:32], ps_qv[0:64])
                    nc.scalar.copy(bdq[64:128, cs, :, 32:64], ps_qv[64:128])

            # ---- prologue: xt right behind wk so K(0) starts early ----
            load_xt(0)
            nc.sync.dma_start(wq_t, wqT.rearrange("(dc p) e -> p dc e", p=128))
            nc.sync.dma_start(wv_t, wvT.rearrange("(dc p) e -> p dc e", p=128))
            nc.sync.dma_start(wo_t, woT.rearrange("(ec p) o -> p ec o", p=128))
            nc.sync.dma_start(pm_t, pmt_r)
            load_mn(0)
            kproj(0)
            qproj(0)

            for g in range(NG):
                xt = xts.pop(g)
                kf = kfs.pop(g)
                mn = mns.pop(g)
                bdq = bdqs[g % 2]

                # ---- logits + mask + exp -> bde ----
                bde = bdep.tile([128, 4, 8, 64], BF16, tag="bde", name=f"bde{g}")
                for c in range(4):
                    ps_l = pslp.tile([128, 8, 64], F32, tag="psl", name="ps_l")
                    for g2 in range(8):
                        nc.tensor.matmul(
                            ps_l[:, g2, :],
                            kf[:, c, g2 * 128 : (g2 + 1) * 128],
                            bdq[:, c, g2, :],
                            start=True, stop=True,
                        )
                    nc.scalar.activation(
                        bde[:, c, :, :], ps_l, AF.Exp, scale=1.0 / np.sqrt(HD)
                    )
                    # zero masked + cross-batch junk cells on the idle Pool engine
                    nc.gpsimd.tensor_tensor(
                        bde[:, c, :, :], bde[:, c, :, :], mn, ALU.mult
                    )

                if debug and g == 0:
                    nc.sync.dma_start(dbg["dxt"].rearrange("a b -> a b"), xt.rearrange("p a b -> p (a b)"))
                    nc.sync.dma_start(dbg["dkf"].rearrange("a b -> a b"), kf.rearrange("p a b -> p (a b)"))
                    nc.sync.dma_start(dbg["dbdq"].rearrange("a b -> a b"), bdq.rearrange("p a b c -> p (a b c)"))
                    nc.sync.dma_start(dbg["dbde"].rearrange("a b -> a b"), bde.rearrange("p a b c -> p (a b c)"))

                # ---- previous group's output projection (covers extract) ----
                oproj_flush()

                # ---- V projection (overlaps exp on ACT) ----
                vt = vtp.tile([128, 8, 512], BF16, tag="vt", name=f"vt{g}")
                for t8 in range(8):
                    ps_v = psp.tile([128, 512], F32, tag="ps", name="ps_v")
                    for dc in range(4):
                        nc.tensor.matmul(
                            ps_v,
                            xt[:, dc, t8 * 128 : (t8 + 1) * 128],
                            wv_t[:, dc, :],
                            start=(dc == 0), stop=(dc == 3),
                        )
                    if t8 < 4:
                        nc.scalar.copy(vt[:, t8, :], ps_v)
                    else:
                        nc.vector.tensor_copy(vt[:, t8, :], ps_v)

                # ---- softmax denominators (replicated across partitions) ----
                rs = rsp.tile([128, 4, 512], F32, tag="rs", name=f"rs{g}")
                for c in range(4):
                    ps_s = pslp.tile([128, 512], F32, tag="psl", name="ps_s")
                    nc.tensor.matmul(
                        ps_s,
                        ones_t,
                        bde[:, c, :, :].rearrange("p a b -> p (a b)"),
                        start=True, stop=True,
                    )
                    nc.vector.reciprocal_approx_fast(out=rs[:, c, :], in_=ps_s)

                if debug and g == 0:
                    nc.sync.dma_start(dbg["drs"].rearrange("a b -> a b"), rs.rearrange("p a b -> p (a b)"))
                    nc.sync.dma_start(dbg["dvt"].rearrange("a b -> a b"), vt.rearrange("p a b -> p (a b)"))

                # ---- next group's K/Q keep the PE busy while DVE/ACT drain ----
                if g + 1 < NG:
                    load_xt(g + 1)
                    load_mn(g + 1)
                    kproj(g + 1)
                    qproj(g + 1)

                # ---- attn @ V with junk parity blocks; extract diag * rs ----
                ao = aop.tile([128, 4, 256], BF16, tag="ao", name=f"ao{g}")
                ao_v = ao.rearrange("p c (h2 gi x) -> p c h2 gi x", h2=2, gi=4)
                rs_v = rs.rearrange("p c (g2 h2 x) -> p c g2 h2 x", g2=8, h2=2)
                for half in range(2):
                    av = avp.tile([128, 4, 4, 64], F32, tag="av", name="ps_av")
                    for gi in range(4):
                        g2 = half * 4 + gi
                        for c in range(4):
                            nc.tensor.matmul(
                                av[:, gi, c, :],
                                vt[:, g2, c * 128 : (c + 1) * 128],
                                bde[:, c, g2, :],
                                start=True, stop=True,
                            )
                    av_v = av.rearrange("p gi c x -> p c gi x")
                    for P in range(2):
                        psl = slice(P * 64, (P + 1) * 64)
                        nc.vector.tensor_tensor(
                            ao_v[psl, :, half, :, :],
                            av_v[psl, :, :, P * 32 : (P + 1) * 32],
                            rs_v[psl, :, half * 4 : (half + 1) * 4, P, :],
                            ALU.mult,
                        )

                if debug and g == 0:
                    nc.sync.dma_start(dbg["dao"].rearrange("a b -> a b"), ao.rearrange("p a b -> p (a b)"))

                pending_o.append((g, ao))

            oproj_flush()

    nc.finalize()
    return nc


_NC_CACHE = None
RUN_KWARGS = {}
LAST_RESULT = None


def _get_nc():
    global _NC_CACHE
    if _NC_CACHE is None:
        _NC_CACHE = build_nc()
    return _NC_CACHE


def _bf16(x):
    return np.ascontiguousarray(x.astype(ml_dtypes.bfloat16))


def kernel(entities, pre_mask, post_mask, W_in, W_out, b_out):
    entities = np.asarray(entities, dtype=np.float32)
    pre_mask = np.asarray(pre_mask)
    post_mask = np.asarray(post_mask)
    W_in = np.asarray(W_in, dtype=np.float32)
    W_out = np.asarray(W_out, dtype=np.float32)
    b_out = np.asarray(b_out, dtype=np.float32)

    wqT = _bf16(W_in[0:512].T)
    wkT = _bf16(W_in[512:1024].T)
    wvT = _bf16(W_in[1024:1536].T)
    woT = _bf16(W_out.T)

    bp_idx = np.arange(2).reshape(2, 1, 1, 1, 1, 1, 1)
    B_idx = np.arange(2).reshape(1, 1, 1, 1, 1, 2, 1)

    in_maps = []
    for i in range(NCORES):
        bsl = slice(i * BPC, (i + 1) * BPC)
        ent_i = _bf16(entities[bsl].reshape(NTOK, D).T)
        # mneg[(bp,j), (g,g2,P,B,q)]: -1e30 where cross-batch or pre-masked
        pm_i = pre_mask[bsl, :NQ, :]                       # (256, 16, 64)
        pm_r = pm_i.reshape(NG, 8, 2, NQ, NE)              # (g, g2, B, q, j)
        pmx = pm_r.transpose(4, 0, 1, 2, 3)                # (j, g, g2, B, q)
        cond = bp_idx != B_idx                             # (2,1,1,1,1,2,1)
        cond = cond | pmx[None, :, :, :, None, :, :]       # (2,j,g,g2,P,B,q)
        cond = np.broadcast_to(cond, (2, NE, NG, 8, 2, 2, NQ))
        msk_i = _bf16(np.where(cond, 0.0, 1.0).reshape(128, NG * 512))
        pmt_i = np.ascontiguousarray(
            (1.0 - post_mask[bsl].astype(np.float32)).reshape(NQT)
        )
        in_maps.append(
            {
                "ent": ent_i,
                "msk": msk_i,
                "pmt": pmt_i,
                "wqT": wqT,
                "wkT": wkT,
                "wvT": wvT,
                "woT": woT,
            }
        )

    nc = _get_nc()
    res = bass_utils.run_bass_kernel_spmd(
        nc, in_maps, list(range(NCORES)), **RUN_KWARGS
    )
    global LAST_RESULT
    LAST_RESULT = res
    outs = [res.results[i]["out"].reshape(BPC, NQ, 512) for i in range(NCORES)]
    full = np.concatenate(outs, axis=0)
    if b_out.any():
        full = full + b_out[None, None, :]
        full = np.where(post_mask[:, :, None], 0.0, full)
    return full.astype(np.float32)
